# revision 9
# baseline (speedup 1.0000x reference)
"""Trainium2 Bass kernel for nn_AttentionGRU (tree attention-GRU).

Self-contained: accepts FULL inputs, shards across 8 NeuronCores internally,
returns the FULL output (softmax probabilities, shape [4]).

Architecture
------------
Host (numpy):
  * compute dependency levels of the parent DAG (longest-path); reorder
    parents level-contiguously (each level padded to a multiple of 128 slots
    with duplicates of a real parent),
  * build per-core gather index/weight slabs for the embedding phase
    (node-sharded; canonical [partition, (k, block)] slot layout),
  * build per-level child-row index + mask tensors.

Device (SPMD on 8 cores, same program, per-core input slabs):
  1. Embedding: xe[n] = sum_k x_word[n,k] * E^T[x_index[n,k]] via per-column
     indirect DMA gathers (128 rows/call) + DVE broadcast-multiply + strided
     reduce.  Each core computes xe for its 1/8 of nodes.
  2. PE-transpose xe to [64, nodes]; AllGather -> every core holds full xe^T.
  3. Leaf GRU (redundant on all cores) -> node_h rows in DRAM.
  4. Level-batched scan: per level gather child rows from node_h, DVE
     attention (sigmoid logits -> masked softmax -> h_tilde), GRU gates via
     PE matmuls in transposed layout, write h rows back to node_h.
  5. max over parent h, output head + softmax on core 0.
"""

import sys

sys.path.insert(0, "/opt/trn_rl_repo")
sys.path.insert(0, "/opt/trn_rl_repo/concourse")

import numpy as np

import concourse.bass as bass
import concourse.bacc as bacc
import concourse.tile as tile
from concourse import mybir
from concourse.masks import make_identity

F32 = mybir.dt.float32
I32 = mybir.dt.int32
ALU = mybir.AluOpType
AXL = mybir.AxisListType
ACT = mybir.ActivationFunctionType

NCORES = 8
H = 64          # hidden dim
K = 32          # words per node
D = 4           # max children
NCLASS = 4


def _ap(base, dims):
    """Strided AP on the same tensor/partition-range as `base` (an AP),
    with explicit free dims [(step, count), ...] (steps in elements)."""
    return bass.AP(base.tensor, base.offset, [list(base.ap[0]), *[[s, c] for s, c in dims]])


# ---------------------------------------------------------------------------
# host-side planning
# ---------------------------------------------------------------------------

def _plan(x_word, x_index, tree):
    N, Kw = x_index.shape
    P, D1 = tree.shape
    Dd = D1 - 1
    L = N - P
    children = tree[:, :Dd].astype(np.int64)

    # --- dependency levels -------------------------------------------------
    lvl = np.zeros(P, np.int64)
    for i in range(P):
        m = -1
        row = children[i]
        for c in row:
            if c >= L and c - L < i:
                v = lvl[c - L]
                if v > m:
                    m = v
        lvl[i] = m + 1
    nlev = int(lvl.max()) + 1

    order = np.argsort(lvl, kind="stable")      # parents sorted by level
    lv_sizes = np.bincount(lvl, minlength=nlev)

    # --- padded level layout ----------------------------------------------
    # parent slot list: per level, real parents then duplicates of the
    # level's first parent, padded to a multiple of 128.
    par_slots = []          # original parent index per parent-slot
    lv_blocks = []
    pos = 0
    for l in range(nlev):
        n = int(lv_sizes[l])
        real = order[pos:pos + n]
        pos += n
        nb = -(-n // 128)
        pad = nb * 128 - n
        par_slots.extend(real.tolist())
        par_slots.extend([int(real[0])] * pad)
        lv_blocks.append(nb)
    # pad total so (L + NPS) % (128 * NCORES) == 0
    tot = L + len(par_slots)
    rem = (-tot) % (128 * NCORES)
    assert rem % 128 == 0
    extra_blocks = rem // 128
    if extra_blocks:
        par_slots.extend([par_slots[-1]] * rem)
        lv_blocks[-1] += extra_blocks
    par_slots = np.asarray(par_slots, np.int64)
    NPS = len(par_slots)
    NEFF = L + NPS                      # total node slots
    NBT = NEFF // 128                   # total 128-blocks
    NBC = NBT // NCORES                 # blocks per core
    LB = L // 128                       # leaf blocks (L assumed %128==0)
    assert L % 128 == 0

    # first slot of each real parent (duplicates later never referenced)
    first_slot = np.full(P, -1, np.int64)
    for s in range(NPS - 1, -1, -1):
        first_slot[par_slots[s]] = s

    # original node id per slot
    slot_node = np.concatenate([np.arange(L), L + par_slots])

    # node_h row of global slot s (partition-major layout)
    def rows_of(slots):
        return ((slots % 128) * NBT + slots // 128).astype(np.int32)

    # --- embedding gather slabs -------------------------------------------
    # per core: CPC = NBC*K columns; chunked by groups of 4 blocks,
    # within a chunk column c = k*nbc + j.
    chunk_sizes = []
    nb_left = NBC
    while nb_left > 0:
        take = min(4, nb_left)
        chunk_sizes.append(take)
        nb_left -= take
    CPC = NBC * Kw
    gidx = np.zeros((NCORES, 128, CPC), np.int32)
    gw = np.zeros((NCORES, 128, CPC), np.float32)
    p_ar = np.arange(128)
    for c in range(NCORES):
        col0 = 0
        b0 = 0
        for nbc in chunk_sizes:
            for k in range(Kw):
                for j in range(nbc):
                    b = c * NBC + b0 + j
                    s = b * 128 + p_ar                 # global slots
                    o = slot_node[s]                   # original nodes
                    gidx[c, :, col0 + k * nbc + j] = x_index[o, k]
                    gw[c, :, col0 + k * nbc + j] = x_word[o, k]
            col0 += nbc * Kw
            b0 += nbc
    # --- per-level child indices + masks ----------------------------------
    levels = []
    pb0 = 0                                            # parent-block offset
    for l in range(nlev):
        nb = lv_blocks[l]
        ncols = nb * D
        idx_l = np.zeros((128, ncols), np.int32)
        msk_l = np.zeros((128, ncols), np.float32)
        for bl in range(nb):
            ps = (pb0 + bl) * 128 + p_ar               # parent slot index
            pi = par_slots[ps]                         # original parent
            ch = children[pi]                          # [128, D]
            for d in range(Dd):
                cd = ch[:, d]
                valid = cd >= 0
                cslot = np.where(
                    cd < L, np.clip(cd, 0, None),
                    L + np.where(cd >= L, first_slot[np.clip(cd - L, 0, P - 1)], 0),
                )
                cslot = np.where(valid, cslot, 0)
                idx_l[:, bl * D + d] = rows_of(cslot)
                msk_l[:, bl * D + d] = valid.astype(np.float32)
        levels.append(dict(nb=nb, idx=idx_l, mask=msk_l, pb0=pb0))
        pb0 += nb

    return dict(
        L=L, P=P, NEFF=NEFF, NBT=NBT, NBC=NBC, LB=LB, CPC=CPC, Kw=Kw,
        chunk_sizes=chunk_sizes, gidx=gidx, gw=gw, levels=levels,
        nlev=nlev,
    )


# ---------------------------------------------------------------------------
# device program
# ---------------------------------------------------------------------------

def _build(plan, vocab):
    NBT, NBC, LB, CPC, Kw = plan["NBT"], plan["NBC"], plan["LB"], plan["CPC"], plan["Kw"]
    NEFF = plan["NEFF"]
    NLOC = NBC * 128                    # nodes per core
    PBLK = NBT - LB                     # parent blocks
    chunk_sizes = plan["chunk_sizes"]
    levels = plan["levels"]

    nc = bacc.Bacc(None, num_devices=NCORES)

    e_t = nc.dram_tensor("e_t", [vocab, H], F32, kind="ExternalInput")
    gidx = nc.dram_tensor("gidx", [128, CPC], I32, kind="ExternalInput")
    gw = nc.dram_tensor("gw", [128, CPC], F32, kind="ExternalInput")
    wz_e = nc.dram_tensor("wz_e", [H + 1, H], F32, kind="ExternalInput")
    wr_e = nc.dram_tensor("wr_e", [H + 1, H], F32, kind="ExternalInput")
    wh_e = nc.dram_tensor("wh_e", [H + 1, H], F32, kind="ExternalInput")
    wa_e = nc.dram_tensor("wa_e", [H + 1, H], F32, kind="ExternalInput")
    uz_t = nc.dram_tensor("uz_t", [H, H], F32, kind="ExternalInput")
    ur_t = nc.dram_tensor("ur_t", [H, H], F32, kind="ExternalInput")
    uh_t = nc.dram_tensor("uh_t", [H, H], F32, kind="ExternalInput")
    wo_e = nc.dram_tensor("wo_e", [H + 1, NCLASS], F32, kind="ExternalInput")
    lvl_idx_t = []
    lvl_msk_t = []
    for l, lv in enumerate(levels):
        ncols = lv["nb"] * D
        lvl_idx_t.append(nc.dram_tensor(f"lvidx{l}", [128, ncols], I32, kind="ExternalInput"))
        lvl_msk_t.append(nc.dram_tensor(f"lvmsk{l}", [128, ncols], F32, kind="ExternalInput"))

    out_t = nc.dram_tensor("out", [1, NCLASS], F32, kind="ExternalOutput")

    node_h = nc.dram_tensor("node_h", [NEFF, H], F32)                    # Internal
    ag_in = nc.dram_tensor("ag_in", [H, NLOC], F32)                      # Internal
    ag_out = nc.dram_tensor("ag_out", [NCORES, H, NLOC], F32, addr_space="Shared")

    node_h3 = node_h[:].rearrange("(p b) e -> p b e", p=128)             # [128,NBT,64]

    BIGCOLS = max(max(cs * Kw for cs in chunk_sizes),
                  max(lv["nb"] * D for lv in levels), PBLK)

    with tile.TileContext(nc) as tc:
        with (
            tc.tile_pool(name="const", bufs=1) as constp,
            tc.tile_pool(name="small", bufs=2) as small,
            tc.tile_pool(name="lvl", bufs=1) as lvlp,
            tc.tile_pool(name="big", bufs=2) as bigp,
            tc.tile_pool(name="psum", bufs=1, space="PSUM") as psum,
            tc.tile_pool(name="psumt", bufs=2, space="PSUM") as psumt,
        ):
            ident = constp.tile([128, 128], F32, tag="ident")
            make_identity(nc, ident[:])

            # weight tiles
            wts = {}
            for nm, t in [("wz", wz_e), ("wr", wr_e), ("wh", wh_e), ("wa", wa_e)]:
                w = constp.tile([H + 1, H], F32, tag=f"w_{nm}")
                nc.sync.dma_start(out=w[:], in_=t[:])
                wts[nm] = w
            for nm, t in [("uz", uz_t), ("ur", ur_t), ("uh", uh_t)]:
                w = constp.tile([H, H], F32, tag=f"w_{nm}")
                nc.sync.dma_start(out=w[:], in_=t[:])
                wts[nm] = w
            wo = constp.tile([H + 1, NCLASS], F32, tag="w_wo")
            nc.sync.dma_start(out=wo[:], in_=wo_e[:])

            # zero-init parent region of node_h (before scan gathers)
            zt = constp.tile([128, 576], F32, tag="zeros")
            nc.vector.memset(zt[:], 0.0)
            zb0 = LB
            while zb0 < NBT:
                zn = min(576 // H, NBT - zb0)
                nc.sync.dma_start(out=node_h3[:, zb0:zb0 + zn, :], in_=zt[:, :zn * H])
                zb0 += zn

            # gather slabs
            gidx_t = constp.tile([128, CPC], I32, tag="gidx")
            nc.sync.dma_start(out=gidx_t[:], in_=gidx[:])
            gw_t = constp.tile([128, CPC], F32, tag="gw")
            nc.sync.dma_start(out=gw_t[:], in_=gw[:])

            # xe^T for local nodes, assembled here then DMA'd to ag_in
            xeT_loc = constp.tile([H, NLOC], F32, tag="xeT_loc")

            # ---------------- phase 1: embedding ---------------------------
            col0 = 0
            b0 = 0
            for nbc in chunk_sizes:
                cols = nbc * Kw
                g = bigp.tile([128, BIGCOLS * H], F32, tag="big")
                for cc in range(cols):
                    nc.gpsimd.indirect_dma_start(
                        out=g[:, cc * H:(cc + 1) * H],
                        out_offset=None,
                        in_=e_t[:],
                        in_offset=bass.IndirectOffsetOnAxis(
                            ap=gidx_t[:, col0 + cc:col0 + cc + 1], axis=0),
                    )
                gv = g[:, :cols * H]
                # multiply by weights (broadcast over H)
                w_ap = _ap(gw_t[:, col0:col0 + cols],
                           [(nbc, Kw), (1, nbc), (0, H)])
                nc.vector.tensor_tensor(
                    out=gv.rearrange("p (k j e) -> p k j e", k=Kw, j=nbc),
                    in0=gv.rearrange("p (k j e) -> p k j e", k=Kw, j=nbc),
                    in1=w_ap, op=ALU.mult)
                # reduce over k -> xe chunk [128, nbc*64]
                xe_c = small.tile([128, 4 * H], F32, tag="xe_c")
                red_in = _ap(gv, [(H, nbc), (1, H), (nbc * H, Kw)])
                nc.vector.tensor_reduce(
                    out=xe_c[:, :nbc * H], in_=red_in, axis=AXL.X, op=ALU.add)
                # transpose each block -> xeT_loc columns
                for j in range(nbc):
                    pt = psumt.tile([H, 128], F32, tag="pt_t")
                    nc.tensor.transpose(
                        out=pt[:], in_=xe_c[:, j * H:(j + 1) * H], identity=ident[:])
                    nc.vector.tensor_copy(
                        out=xeT_loc[:, (b0 + j) * 128:(b0 + j + 1) * 128], in_=pt[:])
                col0 += cols
                b0 += nbc

            nc.gpsimd.dma_start(out=ag_in[:], in_=xeT_loc[:])
            nc.gpsimd.collective_compute(
                "AllGather", ALU.bypass,
                replica_groups=[list(range(NCORES))],
                ins=[ag_in[:].opt()], outs=[ag_out[:].opt()])

            # parent xe^T resident in SBUF (+ ones row for fused bias);
            # leaf xe^T streamed from ag_out chunkwise during the leaf phase.
            NL = LB * 128
            NPAR = NEFF - NL
            xeT = constp.tile([H + 1, NPAR], F32, tag="xeT")
            dst0 = 0
            for c in range(NCORES):
                lo = max(NL, c * NLOC)
                hi = (c + 1) * NLOC
                if hi <= lo:
                    continue
                nc.sync.dma_start(
                    out=xeT[:H, dst0:dst0 + hi - lo],
                    in_=ag_out[c, :, lo - c * NLOC:hi - c * NLOC])
                dst0 += hi - lo
            nc.vector.memset(xeT[H:H + 1, :], 1.0)

            # ---------------- phase 2: leaves ------------------------------
            leaf_ranges = []
            for c in range(NCORES):
                lo = c * NLOC
                hi = min(NL, (c + 1) * NLOC)
                if hi <= lo:
                    break
                o = lo
                while o < hi:
                    cw = min(512, hi - o)
                    leaf_ranges.append((c, o, cw))
                    o += cw
            for (c, i0, nn) in leaf_ranges:
                rhs = small.tile([H + 1, 512], F32, tag="leafx")
                nc.sync.dma_start(out=rhs[:H, :nn],
                                  in_=ag_out[c, :, i0 - c * NLOC:i0 - c * NLOC + nn])
                nc.vector.memset(rhs[H:H + 1, :nn], 1.0)
                pz = psum.tile([H, 512], F32, tag="pz")
                nc.tensor.matmul(out=pz[:, :nn], lhsT=wts["wz"][:], rhs=rhs[:, :nn],
                                 start=True, stop=True)
                zl = small.tile([H, 512], F32, tag="zl")
                nc.scalar.activation(out=zl[:, :nn], in_=pz[:, :nn], func=ACT.Sigmoid)
                ph = psum.tile([H, 512], F32, tag="ph")
                nc.tensor.matmul(out=ph[:, :nn], lhsT=wts["wh"][:], rhs=rhs[:, :nn],
                                 start=True, stop=True)
                cl = small.tile([H, 512], F32, tag="cl")
                nc.scalar.activation(out=cl[:, :nn], in_=ph[:, :nn], func=ACT.Tanh)
                # leaf_h = (1-z)*c = c - z*c
                nc.vector.tensor_tensor(out=zl[:, :nn], in0=zl[:, :nn],
                                        in1=cl[:, :nn], op=ALU.mult)
                nc.vector.tensor_tensor(out=cl[:, :nn], in0=cl[:, :nn],
                                        in1=zl[:, :nn], op=ALU.subtract)
                hr = small.tile([128, 4 * H], F32, tag="hr")
                ngr = nn // 128
                for gr in range(ngr):
                    pt = psumt.tile([128, H], F32, tag="pt_b")
                    nc.tensor.transpose(
                        out=pt[:], in_=cl[:, gr * 128:(gr + 1) * 128],
                        identity=ident[:H, :H])
                    nc.vector.tensor_copy(out=hr[:, gr * H:(gr + 1) * H], in_=pt[:])
                bb = i0 // 128
                nc.sync.dma_start(out=node_h3[:, bb:bb + ngr, :],
                                  in_=hr[:, :ngr * H])

            # ---------------- phase 3: level scan --------------------------
            for l, lv in enumerate(levels):
                nb = lv["nb"]
                ncols = nb * D
                n = nb * 128
                cstart = lv["pb0"] * 128              # parent-local xeT column
                rhs_l = xeT[:, cstart:cstart + n]

                # attention query Q^T then row-major Q
                qT = lvlp.tile([H, n], F32, tag="qT")
                for c0 in range(0, n, 512):
                    cw = min(512, n - c0)
                    pp = psum.tile([H, 512], F32, tag="pp")
                    nc.tensor.matmul(out=pp[:, :cw], lhsT=wts["wa"][:],
                                     rhs=rhs_l[:, c0:c0 + cw],
                                     start=True, stop=True)
                    nc.scalar.activation(out=qT[:, c0:c0 + cw],
                                         in_=pp[:, :cw], func=ACT.Identity)
                # Q rows [128, nb, 64]
                qr = lvlp.tile([128, nb * H], F32, tag="qr")
                for b in range(nb):
                    pt = psumt.tile([128, H], F32, tag="pt_b")
                    nc.tensor.transpose(out=pt[:],
                                        in_=qT[:, b * 128:(b + 1) * 128],
                                        identity=ident[:H, :H])
                    nc.vector.tensor_copy(out=qr[:, b * H:(b + 1) * H], in_=pt[:])

                # child gather
                li = lvlp.tile([128, ncols], I32, tag="lidx")
                nc.sync.dma_start(out=li[:], in_=lvl_idx_t[l][:])
                lm = lvlp.tile([128, ncols], F32, tag="lmask")
                nc.sync.dma_start(out=lm[:], in_=lvl_msk_t[l][:])
                g = bigp.tile([128, BIGCOLS * H], F32, tag="big")
                for cc in range(ncols):
                    nc.gpsimd.indirect_dma_start(
                        out=g[:, cc * H:(cc + 1) * H],
                        out_offset=None,
                        in_=node_h[:],
                        in_offset=bass.IndirectOffsetOnAxis(ap=li[:, cc:cc + 1], axis=0),
                    )
                gv = g[:, :ncols * H]
                g4 = gv.rearrange("p (b d e) -> p b d e", b=nb, d=D)

                # logits = sum_e q*child
                prod = bigp.tile([128, BIGCOLS * H], F32, tag="big")
                pv = prod[:, :ncols * H]
                q_ap = _ap(qr[:, :nb * H], [(H, nb), (0, D), (1, H)])
                nc.vector.tensor_tensor(
                    out=pv.rearrange("p (b d e) -> p b d e", b=nb, d=D),
                    in0=g4, in1=q_ap, op=ALU.mult)
                logit = lvlp.tile([128, ncols], F32, tag="logit")
                nc.vector.tensor_reduce(
                    out=logit[:], in_=pv.rearrange("p (c e) -> p c e", e=H),
                    axis=AXL.X, op=ALU.add)
                # exp(sigmoid(logit)) * mask
                nc.scalar.activation(out=logit[:], in_=logit[:], func=ACT.Sigmoid)
                nc.scalar.activation(out=logit[:], in_=logit[:], func=ACT.Exp)
                nc.vector.tensor_tensor(out=logit[:], in0=logit[:], in1=lm[:],
                                        op=ALU.mult)
                den = lvlp.tile([128, nb], F32, tag="den")
                nc.vector.tensor_reduce(
                    out=den[:], in_=logit[:].rearrange("p (b d) -> p b d", d=D),
                    axis=AXL.X, op=ALU.add)
                nc.vector.reciprocal(out=den[:], in_=den[:])
                nc.vector.tensor_tensor(
                    out=logit[:].rearrange("p (b d) -> p b d", d=D),
                    in0=logit[:].rearrange("p (b d) -> p b d", d=D),
                    in1=_ap(den[:], [(1, nb), (0, D)]), op=ALU.mult)
                # h_tilde = sum_d attn*child
                a_ap = _ap(logit[:], [(D, nb), (1, D), (0, H)])
                nc.vector.tensor_tensor(
                    out=pv.rearrange("p (b d e) -> p b d e", b=nb, d=D),
                    in0=g4, in1=a_ap, op=ALU.mult)
                ht = lvlp.tile([128, nb * H], F32, tag="ht")
                nc.vector.tensor_reduce(
                    out=ht[:], in_=_ap(pv, [(D * H, nb), (1, H), (H, D)]),
                    axis=AXL.X, op=ALU.add)

                # transpose h_tilde -> [64, n]
                htT = lvlp.tile([H, n], F32, tag="htT")
                for b in range(nb):
                    pt = psumt.tile([H, 128], F32, tag="pt_t")
                    nc.tensor.transpose(out=pt[:], in_=ht[:, b * H:(b + 1) * H],
                                        identity=ident[:])
                    nc.vector.tensor_copy(out=htT[:, b * 128:(b + 1) * 128], in_=pt[:])

                # gates (transposed layout); W@xe+b and U@x accumulate in PSUM
                hTt = bigp.tile([128, BIGCOLS * H], F32, tag="big")
                rh = lvlp.tile([H, 512], F32, tag="rh")
                for c0 in range(0, n, 512):
                    cw = min(512, n - c0)
                    sl = slice(c0, c0 + cw)
                    pz = psum.tile([H, 512], F32, tag="pz")
                    nc.tensor.matmul(out=pz[:, :cw], lhsT=wts["wz"][:],
                                     rhs=rhs_l[:, sl], start=True, stop=False)
                    nc.tensor.matmul(out=pz[:, :cw], lhsT=wts["uz"][:],
                                     rhs=htT[:, sl], start=False, stop=True)
                    zt_ = small.tile([H, 512], F32, tag="zt_")
                    nc.scalar.activation(out=zt_[:, :cw], in_=pz[:, :cw],
                                         func=ACT.Sigmoid)
                    pr = psum.tile([H, 512], F32, tag="ph")
                    nc.tensor.matmul(out=pr[:, :cw], lhsT=wts["wr"][:],
                                     rhs=rhs_l[:, sl], start=True, stop=False)
                    nc.tensor.matmul(out=pr[:, :cw], lhsT=wts["ur"][:],
                                     rhs=htT[:, sl], start=False, stop=True)
                    rt_ = small.tile([H, 512], F32, tag="rt_")
                    nc.scalar.activation(out=rt_[:, :cw], in_=pr[:, :cw],
                                         func=ACT.Sigmoid)
                    nc.vector.tensor_tensor(out=rh[:, :cw], in0=rt_[:, :cw],
                                            in1=htT[:, sl], op=ALU.mult)
                    pc = psum.tile([H, 512], F32, tag="pz")
                    nc.tensor.matmul(out=pc[:, :cw], lhsT=wts["wh"][:],
                                     rhs=rhs_l[:, sl], start=True, stop=False)
                    nc.tensor.matmul(out=pc[:, :cw], lhsT=wts["uh"][:],
                                     rhs=rh[:, :cw], start=False, stop=True)
                    ct_ = small.tile([H, 512], F32, tag="ct_")
                    nc.scalar.activation(out=ct_[:, :cw], in_=pc[:, :cw],
                                         func=ACT.Tanh)
                    # h = c + z*(ht - c)
                    nc.vector.tensor_tensor(out=hTt[:H, sl], in0=htT[:, sl],
                                            in1=ct_[:, :cw], op=ALU.subtract)
                    nc.vector.tensor_tensor(out=hTt[:H, sl], in0=hTt[:H, sl],
                                            in1=zt_[:, :cw], op=ALU.mult)
                    nc.vector.tensor_tensor(out=hTt[:H, sl], in0=hTt[:H, sl],
                                            in1=ct_[:, :cw], op=ALU.add)
                # transpose back + write node_h
                hrow = lvlp.tile([128, nb * H], F32, tag="hrow")
                for b in range(nb):
                    pt = psumt.tile([128, H], F32, tag="pt_b")
                    nc.tensor.transpose(out=pt[:], in_=hTt[:H, b * 128:(b + 1) * 128],
                                        identity=ident[:H, :H])
                    nc.vector.tensor_copy(out=hrow[:, b * H:(b + 1) * H], in_=pt[:])
                bb = LB + lv["pb0"]
                nc.sync.dma_start(out=node_h3[:, bb:bb + nb, :], in_=hrow[:, :nb * H])

            # ---------------- phase 4: max + output head -------------------
            hall = bigp.tile([128, BIGCOLS * H], F32, tag="big")
            nc.sync.dma_start(out=hall[:, :PBLK * H], in_=node_h3[:, LB:NBT, :])
            m1 = small.tile([128, H], F32, tag="m1")
            nc.vector.tensor_reduce(
                out=m1[:], in_=_ap(hall[:, :PBLK * H], [(1, H), (H, PBLK)]),
                axis=AXL.X, op=ALU.max)
            pt = psumt.tile([H, 128], F32, tag="pt_t")
            nc.tensor.transpose(out=pt[:], in_=m1[:], identity=ident[:])
            mt = small.tile([H, 128], F32, tag="mt")
            nc.vector.tensor_copy(out=mt[:], in_=pt[:])
            fin = small.tile([H + 1, 1], F32, tag="fin")
            nc.vector.tensor_reduce(out=fin[:H, :], in_=mt[:], axis=AXL.X, op=ALU.max)
            nc.vector.memset(fin[H:H + 1, :], 1.0)
            po = psum.tile([NCLASS, 1], F32, tag="po")
            nc.tensor.matmul(out=po[:], lhsT=wo[:], rhs=fin[:], start=True, stop=True)
            s4 = small.tile([NCLASS, 1], F32, tag="s4")
            nc.vector.tensor_copy(out=s4[:], in_=po[:])
            s4t = small.tile([1, NCLASS], F32, tag="s4t")
            nc.sync.dma_start(out=s4t[:], in_=s4[:])
            nc.scalar.activation(out=s4t[:], in_=s4t[:], func=ACT.Exp)
            ssum = small.tile([1, 1], F32, tag="ssum")
            nc.vector.tensor_reduce(out=ssum[:], in_=s4t[:], axis=AXL.X, op=ALU.add)
            nc.vector.reciprocal(out=ssum[:], in_=ssum[:])
            nc.vector.tensor_tensor(out=s4t[:], in0=s4t[:],
                                    in1=_ap(ssum[:], [(0, NCLASS)]), op=ALU.mult)
            nc.sync.dma_start(out=out_t[:], in_=s4t[:])

    nc.compile()
    return nc


# ---------------------------------------------------------------------------
# entry point
# ---------------------------------------------------------------------------

def _prepare(inputs):
    x_word = np.asarray(inputs["x_word"], np.float32)
    x_index = np.asarray(inputs["x_index"], np.int32)
    tree = np.asarray(inputs["tree"], np.int32)
    E = np.asarray(inputs["E_bu"], np.float32)
    vocab = E.shape[1]

    plan = _plan(x_word, x_index, tree)
    nc = _build(plan, vocab)

    def ext(Wt, b):
        # lhsT [H+1, H]: rows 0..H-1 = W.T, row H = bias
        return np.vstack([np.asarray(Wt, np.float32),
                          np.asarray(b, np.float32).reshape(1, -1)])

    shared = {
        "e_t": np.ascontiguousarray(E.T),
        "wz_e": ext(np.asarray(inputs["W_z_bu"]).T, inputs["b_z_bu"]),
        "wr_e": ext(np.asarray(inputs["W_r_bu"]).T, inputs["b_r_bu"]),
        "wh_e": ext(np.asarray(inputs["W_h_bu"]).T, inputs["b_h_bu"]),
        "wa_e": ext(np.asarray(inputs["W_attn"]), np.zeros(H)),
        "uz_t": np.ascontiguousarray(np.asarray(inputs["U_z_bu"], np.float32).T),
        "ur_t": np.ascontiguousarray(np.asarray(inputs["U_r_bu"], np.float32).T),
        "uh_t": np.ascontiguousarray(np.asarray(inputs["U_h_bu"], np.float32).T),
        "wo_e": np.vstack([np.asarray(inputs["W_out_bu"], np.float32).T,
                           np.asarray(inputs["b_out_bu"], np.float32).reshape(1, -1)]),
    }
    for l, lv in enumerate(plan["levels"]):
        shared[f"lvidx{l}"] = lv["idx"]
        shared[f"lvmsk{l}"] = lv["mask"]

    in_maps = []
    for c in range(NCORES):
        m = dict(shared)
        m["gidx"] = plan["gidx"][c]
        m["gw"] = plan["gw"][c]
        in_maps.append(m)
    return nc, in_maps


def kernel(**inputs) -> np.ndarray:
    from concourse.bass_utils import run_bass_kernel_spmd
    nc, in_maps = _prepare(inputs)
    res = run_bass_kernel_spmd(nc, in_maps, core_ids=list(range(NCORES)))
    return res.results[0]["out"].reshape(NCLASS).astype(np.float32)
